# revision 32
# baseline (speedup 1.0000x reference)
"""Laplace attention kernel for Trainium2 (8 NeuronCores, SPMD data-parallel).

Reference computation (per batch b):
    unnorm[i,j] = sum_d |(k[j,d] - v[i,d]) * 0.5|
    weights     = softmax_j(unnorm)          # rows i, softmax over j
    out[i,:]    = sum_j weights[i,j] * v[j,:]

B=8 batches -> one batch per NeuronCore, no cross-core communication.

Per-core algorithm (M=512, D=64, P=128):
  Index mapping (chosen so every DMA moves 1KB-contiguous blocks per
  partition -> 128 fat descriptors instead of 512 thin ones):
     v/k DRAM row r lives at partition p = r//4, slot s = r%4.
     j(g, l) = 4*l + g   (bank g in 0..3, l = PSUM row 0..127)
     i'(g, p) = g*128 + p (vT2/unnT free-dim column order)
  - Layouts: vT2 [128=(t,d), 512=i'] fp16 : v transposed, duplicated over t
             k2T [128=(t,d), 4, 2, 32] f32 : column (g,W,c) =
                 [k[4(64W+c)+g]; k[4(64W+c+32)+g]]
  - For each bank g, half-window W, pair c: one DVE tensor_scalar
        absd[(t,d), i'] = max(vT2, k2T[:,g,W,c]) = max(v[i',d], k[j_t,d])
    with j_t = 4*(64W + c + 32t) + g; then one TensorE matmul into the
    64-row window unnT_g[64W:64W+64] with the constant selector slice
    band[:, 64-c : 128-c] whose column y is 1 on the 64 d-rows of half t
    exactly when y = c + 32t:
        unnT_g[64W + c + 32t, i'] += sum_d absd[(t,d), i']
    |a-b| = 2*max(a,b) - a - b; the V1[i] part cancels in the softmax and
    the K1[j] part folds into the exp bias.  unnT is produced TRANSPOSED
    ([j, i']) which is exactly the lhsT the final matmul needs, and with
    j = 4*l + g the bias column for bank g is just k1m[:, g] (no scatter).
  - Producer split: banks 0..2 on VectorE (tensor_scalar max), bank 3 on
    ScalarE as Relu(v - k) = max(v,k) - k (bias absorbs the K1 sign flip).
    PE stream: 64 groups of (3 DVE-fed + 1 ScalarE-prebuffered) matmuls.
  - Softmax numerators wT[l, i'] = exp(unnT -+ 0.5*K1 - SHIFT) in bf16.
  - Final matmul with v augmented by a ones column gives numerator and
    denominator together (v_aug_g[p] = v[4p+g] matches wts rows j=4l+g);
    per-half reciprocal + multiply; fat DMA out.

Edge scheduling: fat input DMAs at top priority (v column-halves on the
two HWDGE queues, k whole on the gpsimd SWDGE path); ScalarE act-table
load triggered early; k transposed in f32 straight from the DMA tile (no
cast on the critical path); vT2 duplication copies split DVE/ScalarE;
extra PE warm-up matmuls bridge the transpose->first-main gap so the
p-state never drops back to mid (which costs ~210ns on each of the
first ~12 matmuls).

History: 72.2us naive -> 70.0us (v1: saturated stream, thin DMA) ->
76.9us (v2: fat DMA but p-state regression + slow startup chain) ->
this version.  The ~55us main stream is at the fp16 PE rhs-streaming
floor (512 cols x 1 cycle per matmul, 216ns/mm measured); fp8
DoubleRow (2x PE) was simulated end-to-end and FAILS accuracy
(absmax/scale 0.14-0.33 vs 2e-2: the softmax is near-argmax, so e4m3
logit error of ~0.1-0.3 swaps winners).  uint8/int8 fail the same way.
"""

import os

import numpy as np

M = 512
D = 64
B = 8
P = 128
NB = M // P  # 4 banks
# Global shift on the softmax logits: weights are stored as
# exp(logit - EXP_SHIFT); numerator and denominator scale identically.
EXP_SHIFT = 38.0

_CACHE = {}

CFG = {"mx_dt": "float16"}


def _build_module(cfg=None):
    import concourse.mybir as mybir
    import concourse.tile as tile
    from concourse import bacc

    nc = bacc.Bacc("TRN2", target_bir_lowering=False, debug=False,
                   enable_asserts=False)
    k_dram = nc.dram_tensor("k", [M, D], mybir.dt.float32, kind="ExternalInput")
    v_dram = nc.dram_tensor("v", [M, D], mybir.dt.float32, kind="ExternalInput")
    out_dram = nc.dram_tensor("out", [M, D], mybir.dt.float32,
                              kind="ExternalOutput")
    dbg = None
    if os.environ.get("KERNEL_DEBUG"):
        dbg = {
            "vT2": nc.dram_tensor("vT2_dbg", [M, M], mybir.dt.float32,
                                  kind="ExternalOutput"),
            "k2T": nc.dram_tensor("k2T_dbg", [P, M // 2], mybir.dt.float32,
                                  kind="ExternalOutput"),
            "unn": nc.dram_tensor("unn_dbg", [NB, P, M], mybir.dt.float32,
                                  kind="ExternalOutput"),
        }

    with tile.TileContext(nc) as tc:
        _emit(tc, nc, k_dram.ap(), v_dram.ap(), out_dram.ap(), cfg or CFG,
              dbg=dbg)
    nc.compile()
    return nc


def _emit(tc, nc, k, v, out, cfg, dbg=None):
    from contextlib import ExitStack

    import concourse.mybir as mybir
    from concourse.masks import make_identity

    f32 = mybir.dt.float32
    fp16 = getattr(mybir.dt, cfg.get("mx_dt", "float16"))
    bf16 = mybir.dt.bfloat16
    Alu = mybir.AluOpType
    Act = mybir.ActivationFunctionType

    ctx = ExitStack()
    const = ctx.enter_context(tc.tile_pool(name="const", bufs=1))
    # Deep rings: DVE produces at ~262 ns/tile, PE consumes at ~216; buffering
    # lets the PE run at its native rate.  ScalarE prebuffers all of bank 3.
    absd_pool = ctx.enter_context(tc.tile_pool(name="absd", bufs=48))
    act_pool = ctx.enter_context(tc.tile_pool(name="absd_act", bufs=66))
    wt_pool = ctx.enter_context(tc.tile_pool(name="wt", bufs=4))
    small = ctx.enter_context(tc.tile_pool(name="small", bufs=1))
    psum_tr = ctx.enter_context(tc.tile_pool(name="psum_tr", bufs=1,
                                             space="PSUM"))
    # warm/filler matmuls get their own bank: sharing psum_tr would make
    # the transpose-pool consumers (the vT2/k2T copies) wait on the
    # fillers through pool dependency tracking (~1.8us stall observed).
    psum_warm = ctx.enter_context(tc.tile_pool(name="psum_warm", bufs=1,
                                               space="PSUM"))

    # ---- input DMAs first, at top scheduling priority --------------------
    # Fat descriptors: partition p <- DRAM rows 4p..4p+3 (1KB contiguous).
    # v split by column halves (2 rows = 512B per descriptor) across the
    # two HWDGE queues so cast/transpose can pipeline; k whole on the
    # gpsimd SWDGE path (128 fat descriptors).
    v4 = const.tile([P, NB, D], f32, name="v4")
    k4 = const.tile([P, NB, D], f32, name="k4")
    v_view = v.rearrange("(p s) d -> p s d", p=P)
    k_view = k.rearrange("(p s) d -> p s d", p=P)
    with tc.high_priority():
        nc.gpsimd.dma_start(k4[:], k_view[:])
        nc.sync.dma_start(v4[:, 0:2, :], v_view[:, 0:2, :])
        nc.scalar.dma_start(v4[:, 2:4, :], v_view[:, 2:4, :])

    # ---- static tensors (gpsimd, after the DMA issues) -------------------
    warm_src = const.tile([P, P], fp16, name="warm_src")
    nc.gpsimd.memset(warm_src[:], 0.0)
    # Trigger the ~1.3us ScalarE ACT_TABLE_LOAD while the DMAs are in
    # flight so it is off the startup critical path.
    act_warm = const.tile([P, 1], fp16, name="act_warm")
    nc.scalar.activation(act_warm[:], warm_src[:, 0:1], Act.Copy)
    ident32 = const.tile([P, P], f32, name="ident32")
    make_identity(nc, ident32)
    # band[(t,d), y] = 1 iff y == 64 + 32*t.  Slice [64-c : 128-c] puts the
    # t=0 ones at column c and the t=1 ones at column c+32.
    band = const.tile([P, P], fp16, name="band")
    nc.gpsimd.memset(band[:], 0.0)
    nc.gpsimd.memset(band[0:D, D:D + 1], 1.0)
    nc.gpsimd.memset(band[D:2 * D, D + 32:D + 33], 1.0)

    # ---- PE p-state warmup while DMAs are in flight ----------------------
    warm = psum_warm.tile([1, P], f32, name="warm", tag="warm")
    for _ in range(4):
        nc.tensor.matmul(warm[:], warm_src[:, 0:1], warm_src[:, 0:P],
                         start=True, stop=True)

    # ---- vT2 [128=(t,d), 512=i'] fp16 ------------------------------------
    # Per column-half c: PE-transpose straight from the f32 DMA tile (no
    # cast step on the critical path); ptrv[(b,d), c*128+p] = v[4p+2c+b, d].
    # Column group g = 2c+b of vT2 copies block (c,b) with the fp16 cast
    # folded into the PSUM->SBUF copy.  i'(g,p) = g*128 + p <-> v row 4p+g.
    vT2 = const.tile([P, M], fp16, name="vT2")
    ptrv = psum_tr.tile([P, 2 * P], f32, name="ptrv", tag="ptrv")
    for c in range(2):
        nc.tensor.transpose(ptrv[:, c * P:(c + 1) * P],
                            v4[:, 2 * c:2 * c + 2, :].rearrange(
                                "p s d -> p (s d)"),
                            ident32[:])
    # ---- k transposes (f32, straight from the DMA tile) ------------------
    # ptrk[(b,d), c*128+p] = k[4p+2c+b, d]
    ptrk = psum_tr.tile([P, 2 * P], f32, name="ptrk", tag="ptrk")
    for c in range(2):
        nc.tensor.transpose(ptrk[:, c * P:(c + 1) * P],
                            k4[:, 2 * c:2 * c + 2, :].rearrange(
                                "p s d -> p (s d)"),
                            ident32[:])
    # p-state fillers that DEPEND on v4 (so the scheduler cannot hoist
    # them ahead of the DMA, or between the transposes): they keep the PE
    # busy after the transposes while the copies run (an idle PE drops to
    # the mid p-state, costing ~210ns on each of the first ~10 matmuls).
    for _ in range(3):
        nc.tensor.matmul(warm[:, 0:D], v4[:, 0, 0:1], v4[:, 0, :],
                         start=True, stop=True)

    # vT2 assembly: dst col = g*128 + p, g = 2c+b; copies (b, t).
    # t=0 halves on DVE, t=1 on ScalarE, so the two chains run in parallel.
    def vT2_copy(eng, b, t):
        src = ptrv[b * D:(b + 1) * D, :].rearrange("d (c p) -> d c p", c=2)
        dst = vT2[t * D:(t + 1) * D, :].rearrange(
            "d (c b2 p) -> d b2 c p", c=2, b2=2)[:, b, :, :]
        (eng.tensor_copy if eng is nc.vector else eng.copy)(dst, src)

    vT2_copy(nc.vector, 0, 0)
    vT2_copy(nc.vector, 1, 0)
    vT2_copy(nc.scalar, 0, 1)
    vT2_copy(nc.scalar, 1, 1)

    # ---- k2T [128=(t,d), 4, 2, 32] f32 -----------------------------------
    # Column (g, W, c): t=0 -> k[4(64W+c)+g] (block-g col 64W+c),
    #                   t=1 -> k[4(64W+c+32)+g] (block-g col 64W+c+32).
    # Block-g col p decomposes p = 64W + 32t + c.  b0 copies on DVE (its
    # banks 0..2 need them first), b1 + neg on ScalarE (feeds its bank 3).
    k2T = const.tile([P, NB, 2, 32], f32, name="k2T")

    def k2T_copy(eng, b, t):
        src = ptrk[b * D:(b + 1) * D, :].rearrange(
            "d (c w tt cc) -> d tt c w cc", c=2, w=2, tt=2)[:, t, :, :, :]
        dst = k2T[t * D:(t + 1) * D, :, :, :].rearrange(
            "d (c b2) w cc -> d b2 c w cc", c=2, b2=2)[:, b, :, :, :]
        (eng.tensor_copy if eng is nc.vector else eng.copy)(dst, src)

    k2T_copy(nc.vector, 0, 0)
    k2T_copy(nc.vector, 0, 1)
    k2T_copy(nc.scalar, 1, 0)
    k2T_copy(nc.scalar, 1, 1)
    # Bank-3 negated columns for the ScalarE Relu bias.
    neg_k2T = const.tile([P, D], f32, name="neg_k2T")
    nc.scalar.mul(neg_k2T[:].rearrange("p (w cc) -> p w cc", w=2),
                  k2T[:, 3, :, :], -1.0)
    # p-state fillers that depend on vT2 (the last startup dependency of
    # the main stream): they run right before the first main matmul.
    for _ in range(3):
        nc.tensor.matmul(warm[:], vT2[:, 0:1], vT2[:, 0:P],
                         start=True, stop=True)

    # ---- K1[j] = sum_d k[j,d]: k1m[p, s] = K1[4p+s] ----------------------
    # Bank g's bias column is exactly k1m[:, g] (j = 4*l + g).
    k1m = const.tile([P, NB], f32, name="k1m")
    k1scr = const.tile([P, D], fp16, name="k1scr")

    def emit_k1():
        for s in range(NB):
            nc.scalar.activation(k1scr[:], k4[:, s, :], Act.Copy,
                                 accum_out=k1m[:, s:s + 1])

    # ---- main-phase PSUM pools -------------------------------------------
    psum_unn = ctx.enter_context(tc.tile_pool(name="psum_unn", bufs=4,
                                              space="PSUM"))
    psum_out = ctx.enter_context(tc.tile_pool(name="psum_out", bufs=1,
                                              space="PSUM"))
    out_all = psum_out.tile([P, NB, D + 1], f32, name="out_all")
    unns = [None] * NB
    for g in range(NB):
        unns[g] = psum_unn.tile([P, M], f32, name=f"unn_{g}", tag="unn")

    bias_col = [None] * NB
    wts = [None] * NB

    def emit_bias():
        for g in range(NB):
            bc = const.tile([P, 1], f32, name=f"bias_{g}")
            sgn = 0.5 if g == NB - 1 else -0.5  # bank 3 is the Relu path
            nc.scalar.activation(bc[:], k1m[:, g:g + 1], Act.Copy,
                                 bias=-EXP_SHIFT, scale=sgn)
            bias_col[g] = bc

    v_aug = []

    def emit_v_aug():
        for g in range(NB):
            va = const.tile([P, D + 1], bf16, name=f"v_aug_{g}")
            nc.scalar.copy(va[:, 0:D], v4[:, g, :])
            nc.gpsimd.memset(va[:, D:D + 1], 1.0)
            v_aug.append(va)

    def emit_exp(g, chunks=1):
        wT = wt_pool.tile([P, M], bf16, name="wT", tag="wT")
        wts[g] = wT
        cw = M // chunks
        for c in range(chunks):
            nc.scalar.activation(wT[:, c * cw:(c + 1) * cw],
                                 unns[g][:, c * cw:(c + 1) * cw],
                                 Act.Exp, scale=1.0, bias=bias_col[g][:])

    # ---- bank-3 distance tiles: ScalarE emits all 64 up front, with the
    # drain-phase helpers slotted into its queue where they have slack ----
    absd_a_tiles = {}
    for step in range(64):
        # consumption order ss -> (w, c) = (ss%2, ss//2); neg col = w*32+c
        nidx = (step % 2) * 32 + step // 2
        absd = act_pool.tile([P, M], fp16, name="absd_a", tag="absd_a")
        nc.scalar.activation(absd[:], vT2[:], Act.Relu,
                             bias=neg_k2T[:, nidx:nidx + 1], scale=1.0)
        absd_a_tiles[step] = absd
        # tile_wait_until keeps the list scheduler from hoisting these
        # ready-input helpers onto ScalarE ahead of the critical
        # vT2/k2T startup copies.
        if step == 3:
            with tc.tile_wait_until(0.0060):
                emit_v_aug()
        elif step == 6:
            with tc.tile_wait_until(0.0060):
                emit_k1()
        elif step == 16:
            with tc.tile_wait_until(0.0065):
                emit_bias()

    def emit_step2(g, w, c, absd):
        nc.tensor.matmul(
            unns[g][w * D:(w + 1) * D, :], band[:, D - c:2 * D - c],
            absd[:], start=(c == 0), stop=(c == 31), skip_group_check=True)

    # PE stream: groups of (3 VectorE-fed + 1 ScalarE-prebuffered) matmuls.
    # First group is all-DVE (ScalarE's first Relu tile lands late in the
    # startup chain).  The tail is 20 DVE steps then 8 ScalarE-prebuffered
    # steps: bank 2 (the last DVE bank) closes ~1.7us before the stream
    # ends, so its exp and the g<3 output matmuls overlap the final
    # ScalarE-fed matmuls; only bank 3's exp trails the stream.
    sched = ["D"] * 4
    for gi in range(56):
        sched += ["D", "D", "D", "S"]
    sched += ["D"] * 20
    sched += ["S"] * 8
    ds = 0
    ss = 0
    # w alternates every step so consecutive matmuls write disjoint PSUM
    # partition halves and can pipeline in the PE (the v1 kernel's
    # h-alternation measured ~210ns/mm effective vs 216 serialized).
    for kind in sched:
        if kind == "D":
            g, step = ds // 64, ds % 64
            ds += 1
            w, c = step % 2, step // 2
            absd = absd_pool.tile([P, M], fp16, name="absd", tag="absd")
            nc.vector.tensor_scalar(
                absd[:], vT2[:], k2T[:, g, w, c:c + 1], None, op0=Alu.max)
            emit_step2(g, w, c, absd)
        else:
            w, c = ss % 2, ss // 2
            emit_step2(NB - 1, w, c, absd_a_tiles[ss])
            ss += 1

    # ---- softmax numerators ----------------------------------------------
    # Banks 0..2 close inside the stream so their exps overlap it; bank
    # 3's exp is the only one that trails, chunked by qp-halves so the
    # first g=3 output matmuls start after half the exp.
    emit_exp(0)
    emit_exp(1)
    emit_exp(2)
    emit_exp(3, chunks=2)

    # ---- weighted sum + denominator via augmented-ones column ------------
    # g = 3 accumulated last within each group: bank 3's exp is the only
    # one that trails the stream, so 12 of the 16 matmuls run before it.
    for qp in range(NB):
        for g in (0, 1, 2, 3):
            nc.tensor.matmul(
                out_all[:, qp, :], wts[g][:, qp * P:(qp + 1) * P],
                v_aug[g][:], start=(g == 0), stop=(g == 3),
                skip_group_check=True)

    # ---- normalize per half + fat DMA out --------------------------------
    # res[p, s, :] = out row 4p+s -> 1KB contiguous per partition.
    recip = small.tile([P, NB], f32, name="recip")
    res = small.tile([P, NB, D], f32, name="res")
    out_v = out.rearrange("(p s) d -> p s d", p=P)
    for h in range(2):
        sl = slice(2 * h, 2 * h + 2)
        nc.vector.reciprocal(recip[:, sl], out_all[:, sl, D])
        rb = recip[:, sl].unsqueeze(2).broadcast_to((P, 2, D))
        nc.vector.tensor_tensor(res[:, sl, :], out_all[:, sl, 0:D], rb,
                                op=Alu.mult)
        nc.sync.dma_start(out_v[:, sl, :], res[:, sl, :])

    if dbg is not None:
        vT2f = small.tile([P, M], f32, name="vT2f")
        nc.vector.tensor_copy(vT2f[:], vT2[:])
        nc.sync.dma_start(dbg["vT2"].ap()[0:P, :], vT2f[:])
        nc.sync.dma_start(dbg["k2T"].ap(),
                          k2T[:].rearrange("p g w c -> p (g w c)"))
        for g in range(NB):
            ut = small.tile([P, M], f32, name=f"unn_dbg_{g}")
            nc.vector.tensor_copy(ut[:], unns[g][:])
            nc.sync.dma_start(dbg["unn"].ap()[g], ut[:])

    ctx.close()


def _get_module():
    if "nc" not in _CACHE:
        _CACHE["nc"] = _build_module()
    return _CACHE["nc"]


def _run(k, v, trace=False, tmpdir=None):
    """k, v: [B, M, D] f32. Returns (out [B, M, D] f32, BassKernelResults)."""
    from concourse import bass_utils

    nc = _get_module()
    kw = {"tmpdir": tmpdir} if tmpdir else {}
    in_maps = [
        {"k": np.ascontiguousarray(k[b], dtype=np.float32),
         "v": np.ascontiguousarray(v[b], dtype=np.float32)}
        for b in range(B)
    ]
    res = bass_utils.run_bass_kernel_spmd(
        nc, in_maps, core_ids=list(range(B)), trace=trace, **kw)
    out = np.stack([res.results[b]["out"] for b in range(B)], axis=0)
    return out, res


def kernel(**inputs):
    k = np.asarray(inputs["k"])
    v = np.asarray(inputs["v"])
    trace = bool(int(os.environ.get("KERNEL_TRACE", "0")))
    try:
        out, _ = _run(k, v, trace=trace)
    except Exception:
        # transient device hiccups happen; one retry on a fresh attempt
        out, _ = _run(k, v, trace=trace)
    return out.astype(np.float32)


# revision 33
# speedup vs baseline: 1.0121x; 1.0121x over previous
"""Laplace attention kernel for Trainium2 (8 NeuronCores, SPMD data-parallel).

Reference computation (per batch b):
    unnorm[i,j] = sum_d |(k[j,d] - v[i,d]) * 0.5|
    weights     = softmax_j(unnorm)          # rows i, softmax over j
    out[i,:]    = sum_j weights[i,j] * v[j,:]

B=8 batches -> one batch per NeuronCore, no cross-core communication.

Per-core algorithm (M=512, D=64, P=128):
  Index mapping (chosen so every DMA moves 1KB-contiguous blocks per
  partition -> 128 fat descriptors instead of 512 thin ones):
     v/k DRAM row r lives at partition p = r//4, slot s = r%4.
     j(g, l) = 4*l + g   (bank g in 0..3, l = PSUM row 0..127)
     i'(g, p) = g*128 + p (vT2/unnT free-dim column order)
  - Layouts: vT2 [128=(t,d), 512=i'] fp16 : v transposed, duplicated over t
             k2T [128=(t,d), 4, 2, 32] f32 : column (g,W,c) =
                 [k[4(64W+c)+g]; k[4(64W+c+32)+g]]
  - For each bank g, half-window W, pair c: one DVE tensor_scalar
        absd[(t,d), i'] = max(vT2, k2T[:,g,W,c]) = max(v[i',d], k[j_t,d])
    with j_t = 4*(64W + c + 32t) + g; then one TensorE matmul into the
    64-row window unnT_g[64W:64W+64] with the constant selector slice
    band[:, 64-c : 128-c] whose column y is 1 on the 64 d-rows of half t
    exactly when y = c + 32t:
        unnT_g[64W + c + 32t, i'] += sum_d absd[(t,d), i']
    |a-b| = 2*max(a,b) - a - b; the V1[i] part cancels in the softmax and
    the K1[j] part folds into the exp bias.  unnT is produced TRANSPOSED
    ([j, i']) which is exactly the lhsT the final matmul needs, and with
    j = 4*l + g the bias column for bank g is just k1m[:, g] (no scatter).
  - Producer split: banks 0..2 on VectorE (tensor_scalar max), bank 3 on
    ScalarE as Relu(v - k) = max(v,k) - k (bias absorbs the K1 sign flip).
    PE stream: 64 groups of (3 DVE-fed + 1 ScalarE-prebuffered) matmuls.
  - Softmax numerators wT[l, i'] = exp(unnT -+ 0.5*K1 - SHIFT) in bf16.
  - Final matmul with v augmented by a ones column gives numerator and
    denominator together (v_aug_g[p] = v[4p+g] matches wts rows j=4l+g);
    per-half reciprocal + multiply; fat DMA out.

Edge scheduling: fat input DMAs at top priority (v column-halves on the
two HWDGE queues, k whole on the gpsimd SWDGE path); ScalarE act-table
load triggered early; k transposed in f32 straight from the DMA tile (no
cast on the critical path); vT2 duplication copies split DVE/ScalarE;
extra PE warm-up matmuls bridge the transpose->first-main gap so the
p-state never drops back to mid (which costs ~210ns on each of the
first ~12 matmuls).

History: 72.2us naive -> 70.0us (v1: saturated stream, thin DMA) ->
76.9us (v2: fat DMA but p-state regression + slow startup chain) ->
this version.  The ~55us main stream is at the fp16 PE rhs-streaming
floor (512 cols x 1 cycle per matmul, 216ns/mm measured); fp8
DoubleRow (2x PE) was simulated end-to-end and FAILS accuracy
(absmax/scale 0.14-0.33 vs 2e-2: the softmax is near-argmax, so e4m3
logit error of ~0.1-0.3 swaps winners).  uint8/int8 fail the same way.
"""

import os

import numpy as np

M = 512
D = 64
B = 8
P = 128
NB = M // P  # 4 banks
# Global shift on the softmax logits: weights are stored as
# exp(logit - EXP_SHIFT); numerator and denominator scale identically.
EXP_SHIFT = 38.0

_CACHE = {}

CFG = {"mx_dt": "float16"}


def _build_module(cfg=None):
    import concourse.mybir as mybir
    import concourse.tile as tile
    from concourse import bacc

    nc = bacc.Bacc("TRN2", target_bir_lowering=False, debug=False,
                   enable_asserts=False)
    k_dram = nc.dram_tensor("k", [M, D], mybir.dt.float32, kind="ExternalInput")
    v_dram = nc.dram_tensor("v", [M, D], mybir.dt.float32, kind="ExternalInput")
    out_dram = nc.dram_tensor("out", [M, D], mybir.dt.float32,
                              kind="ExternalOutput")
    dbg = None
    if os.environ.get("KERNEL_DEBUG"):
        dbg = {
            "vT2": nc.dram_tensor("vT2_dbg", [M, M], mybir.dt.float32,
                                  kind="ExternalOutput"),
            "k2T": nc.dram_tensor("k2T_dbg", [P, M // 2], mybir.dt.float32,
                                  kind="ExternalOutput"),
            "unn": nc.dram_tensor("unn_dbg", [NB, P, M], mybir.dt.float32,
                                  kind="ExternalOutput"),
        }

    with tile.TileContext(nc) as tc:
        _emit(tc, nc, k_dram.ap(), v_dram.ap(), out_dram.ap(), cfg or CFG,
              dbg=dbg)
    nc.compile()
    return nc


def _emit(tc, nc, k, v, out, cfg, dbg=None):
    from contextlib import ExitStack

    import concourse.mybir as mybir
    from concourse.masks import make_identity

    f32 = mybir.dt.float32
    fp16 = getattr(mybir.dt, cfg.get("mx_dt", "float16"))
    bf16 = mybir.dt.bfloat16
    Alu = mybir.AluOpType
    Act = mybir.ActivationFunctionType

    ctx = ExitStack()
    const = ctx.enter_context(tc.tile_pool(name="const", bufs=1))
    # Deep rings: DVE produces at ~262 ns/tile, PE consumes at ~216; buffering
    # lets the PE run at its native rate.  ScalarE prebuffers all of bank 3.
    absd_pool = ctx.enter_context(tc.tile_pool(name="absd", bufs=48))
    act_pool = ctx.enter_context(tc.tile_pool(name="absd_act", bufs=66))
    wt_pool = ctx.enter_context(tc.tile_pool(name="wt", bufs=4))
    small = ctx.enter_context(tc.tile_pool(name="small", bufs=1))
    psum_tr = ctx.enter_context(tc.tile_pool(name="psum_tr", bufs=1,
                                             space="PSUM"))
    # warm/filler matmuls get their own bank: sharing psum_tr would make
    # the transpose-pool consumers (the vT2/k2T copies) wait on the
    # fillers through pool dependency tracking (~1.8us stall observed).
    psum_warm = ctx.enter_context(tc.tile_pool(name="psum_warm", bufs=1,
                                               space="PSUM"))

    # ---- input DMAs first, at top scheduling priority --------------------
    # Fat descriptors: partition p <- DRAM rows 4p..4p+3 (1KB contiguous).
    # v split by column halves (2 rows = 512B per descriptor) across the
    # two HWDGE queues so cast/transpose can pipeline; k whole on the
    # gpsimd SWDGE path (128 fat descriptors).
    v4 = const.tile([P, NB, D], f32, name="v4")
    k4 = const.tile([P, NB, D], f32, name="k4")
    v_view = v.rearrange("(p s) d -> p s d", p=P)
    k_view = k.rearrange("(p s) d -> p s d", p=P)
    with tc.high_priority():
        nc.gpsimd.dma_start(k4[:], k_view[:])
        nc.sync.dma_start(v4[:, 0:2, :], v_view[:, 0:2, :])
        nc.scalar.dma_start(v4[:, 2:4, :], v_view[:, 2:4, :])

    # ---- static tensors (gpsimd, after the DMA issues) -------------------
    warm_src = const.tile([P, P], fp16, name="warm_src")
    nc.gpsimd.memset(warm_src[:], 0.0)
    # Trigger the ~1.3us ScalarE ACT_TABLE_LOAD while the DMAs are in
    # flight so it is off the startup critical path.
    act_warm = const.tile([P, 1], fp16, name="act_warm")
    nc.scalar.activation(act_warm[:], warm_src[:, 0:1], Act.Copy)
    ident32 = const.tile([P, P], f32, name="ident32")
    make_identity(nc, ident32)
    # band[(t,d), y] = 1 iff y == 64 + 32*t.  Slice [64-c : 128-c] puts the
    # t=0 ones at column c and the t=1 ones at column c+32.
    band = const.tile([P, P], fp16, name="band")
    nc.gpsimd.memset(band[:], 0.0)
    nc.gpsimd.memset(band[0:D, D:D + 1], 1.0)
    nc.gpsimd.memset(band[D:2 * D, D + 32:D + 33], 1.0)

    # ---- PE p-state warmup while DMAs are in flight ----------------------
    warm = psum_warm.tile([1, P], f32, name="warm", tag="warm")
    for _ in range(4):
        nc.tensor.matmul(warm[:], warm_src[:, 0:1], warm_src[:, 0:P],
                         start=True, stop=True)

    # ---- vT2 [128=(t,d), 512=i'] fp16 ------------------------------------
    # Per column-half c: PE-transpose straight from the f32 DMA tile (no
    # cast step on the critical path); ptrv[(b,d), c*128+p] = v[4p+2c+b, d].
    # Column group g = 2c+b of vT2 copies block (c,b) with the fp16 cast
    # folded into the PSUM->SBUF copy.  i'(g,p) = g*128 + p <-> v row 4p+g.
    vT2 = const.tile([P, M], fp16, name="vT2")
    ptrv = psum_tr.tile([P, 2 * P], f32, name="ptrv", tag="ptrv")
    for c in range(2):
        nc.tensor.transpose(ptrv[:, c * P:(c + 1) * P],
                            v4[:, 2 * c:2 * c + 2, :].rearrange(
                                "p s d -> p (s d)"),
                            ident32[:])
    # ---- k transposes (f32, straight from the DMA tile) ------------------
    # ptrk[(b,d), c*128+p] = k[4p+2c+b, d]
    ptrk = psum_tr.tile([P, 2 * P], f32, name="ptrk", tag="ptrk")
    for c in range(2):
        nc.tensor.transpose(ptrk[:, c * P:(c + 1) * P],
                            k4[:, 2 * c:2 * c + 2, :].rearrange(
                                "p s d -> p (s d)"),
                            ident32[:])
    # p-state fillers that DEPEND on v4 (so the scheduler cannot hoist
    # them ahead of the DMA, or between the transposes): they keep the PE
    # busy after the transposes while the copies run (an idle PE drops to
    # the mid p-state, costing ~210ns on each of the first ~10 matmuls).
    for _ in range(3):
        nc.tensor.matmul(warm[:, 0:D], v4[:, 0, 0:1], v4[:, 0, :],
                         start=True, stop=True)

    # vT2 assembly: dst col = g*128 + p, g = 2c+b; copies (b, t).
    # t=0 halves on DVE, t=1 on ScalarE, so the two chains run in parallel.
    def vT2_copy(eng, b, t):
        src = ptrv[b * D:(b + 1) * D, :].rearrange("d (c p) -> d c p", c=2)
        dst = vT2[t * D:(t + 1) * D, :].rearrange(
            "d (c b2 p) -> d b2 c p", c=2, b2=2)[:, b, :, :]
        (eng.tensor_copy if eng is nc.vector else eng.copy)(dst, src)

    vT2_copy(nc.vector, 0, 0)
    vT2_copy(nc.vector, 1, 0)
    vT2_copy(nc.vector, 0, 1)
    vT2_copy(nc.scalar, 1, 1)

    # ---- k2T [128=(t,d), 4, 2, 32] f32 -----------------------------------
    # Column (g, W, c): t=0 -> k[4(64W+c)+g] (block-g col 64W+c),
    #                   t=1 -> k[4(64W+c+32)+g] (block-g col 64W+c+32).
    # Block-g col p decomposes p = 64W + 32t + c.  b0 copies on DVE (its
    # banks 0..2 need them first), b1 + neg on ScalarE (feeds its bank 3).
    k2T = const.tile([P, NB, 2, 32], f32, name="k2T")

    def k2T_copy(eng, b, t):
        src = ptrk[b * D:(b + 1) * D, :].rearrange(
            "d (c w tt cc) -> d tt c w cc", c=2, w=2, tt=2)[:, t, :, :, :]
        dst = k2T[t * D:(t + 1) * D, :, :, :].rearrange(
            "d (c b2) w cc -> d b2 c w cc", c=2, b2=2)[:, b, :, :, :]
        (eng.tensor_copy if eng is nc.vector else eng.copy)(dst, src)

    k2T_copy(nc.vector, 0, 0)
    k2T_copy(nc.vector, 0, 1)
    k2T_copy(nc.scalar, 1, 0)
    k2T_copy(nc.scalar, 1, 1)
    # Bank-3 negated columns for the ScalarE Relu bias.
    neg_k2T = const.tile([P, D], fp16, name="neg_k2T")
    nc.scalar.mul(neg_k2T[:].rearrange("p (w cc) -> p w cc", w=2),
                  k2T[:, 3, :, :], -1.0)
    # p-state fillers that depend on vT2 (the last startup dependency of
    # the main stream): they run right before the first main matmul.
    for _ in range(3):
        nc.tensor.matmul(warm[:], vT2[:, 0:1], vT2[:, 0:P],
                         start=True, stop=True)

    # ---- K1[j] = sum_d k[j,d]: k1m[p, s] = K1[4p+s] ----------------------
    # Bank g's bias column is exactly k1m[:, g] (j = 4*l + g).
    k1m = const.tile([P, NB], f32, name="k1m")
    k1scr = const.tile([P, D], fp16, name="k1scr")

    def emit_k1():
        for s in range(NB):
            nc.scalar.activation(k1scr[:], k4[:, s, :], Act.Copy,
                                 accum_out=k1m[:, s:s + 1])

    # ---- main-phase PSUM pools -------------------------------------------
    psum_unn = ctx.enter_context(tc.tile_pool(name="psum_unn", bufs=4,
                                              space="PSUM"))
    psum_out = ctx.enter_context(tc.tile_pool(name="psum_out", bufs=1,
                                              space="PSUM"))
    out_all = psum_out.tile([P, NB, D + 1], f32, name="out_all")
    unns = [None] * NB
    for g in range(NB):
        unns[g] = psum_unn.tile([P, M], f32, name=f"unn_{g}", tag="unn")

    bias_col = [None] * NB
    wts = [None] * NB

    def emit_bias():
        for g in range(NB):
            bc = const.tile([P, 1], f32, name=f"bias_{g}")
            sgn = 0.5 if g == NB - 1 else -0.5  # bank 3 is the Relu path
            nc.scalar.activation(bc[:], k1m[:, g:g + 1], Act.Copy,
                                 bias=-EXP_SHIFT, scale=sgn)
            bias_col[g] = bc

    v_aug = []

    def emit_v_aug():
        for g in range(NB):
            va = const.tile([P, D + 1], bf16, name=f"v_aug_{g}")
            nc.scalar.copy(va[:, 0:D], v4[:, g, :])
            nc.gpsimd.memset(va[:, D:D + 1], 1.0)
            v_aug.append(va)

    def emit_exp(g, chunks=1):
        wT = wt_pool.tile([P, M], bf16, name="wT", tag="wT")
        wts[g] = wT
        cw = M // chunks
        for c in range(chunks):
            nc.scalar.activation(wT[:, c * cw:(c + 1) * cw],
                                 unns[g][:, c * cw:(c + 1) * cw],
                                 Act.Exp, scale=1.0, bias=bias_col[g][:])

    # ---- bank-3 distance tiles: ScalarE emits all 64 up front, with the
    # drain-phase helpers slotted into its queue where they have slack ----
    absd_a_tiles = {}
    for step in range(64):
        # consumption order ss -> (w, c) = (ss%2, ss//2); neg col = w*32+c
        nidx = (step % 2) * 32 + step // 2
        absd = act_pool.tile([P, M], fp16, name="absd_a", tag="absd_a")
        nc.scalar.activation(absd[:], vT2[:], Act.Relu,
                             bias=neg_k2T[:, nidx:nidx + 1], scale=1.0)
        absd_a_tiles[step] = absd
        # tile_wait_until keeps the list scheduler from hoisting these
        # ready-input helpers onto ScalarE ahead of the critical
        # vT2/k2T startup copies.
        if step == 3:
            with tc.tile_wait_until(0.0060):
                emit_v_aug()
        elif step == 6:
            with tc.tile_wait_until(0.0060):
                emit_k1()
        elif step == 16:
            with tc.tile_wait_until(0.0065):
                emit_bias()

    def emit_step2(g, w, c, absd):
        nc.tensor.matmul(
            unns[g][w * D:(w + 1) * D, :], band[:, D - c:2 * D - c],
            absd[:], start=(c == 0), stop=(c == 31), skip_group_check=True)

    # PE stream: groups of (3 VectorE-fed + 1 ScalarE-prebuffered) matmuls.
    # First group is all-DVE (ScalarE's first Relu tile lands late in the
    # startup chain).  The tail is 20 DVE steps then 8 ScalarE-prebuffered
    # steps: bank 2 (the last DVE bank) closes ~1.7us before the stream
    # ends, so its exp and the g<3 output matmuls overlap the final
    # ScalarE-fed matmuls; only bank 3's exp trails the stream.
    sched = ["D"] * 4
    for gi in range(56):
        sched += ["D", "D", "D", "S"]
    sched += ["D"] * 20
    sched += ["S"] * 8
    ds = 0
    ss = 0
    # w alternates every step so consecutive matmuls write disjoint PSUM
    # partition halves and can pipeline in the PE (the v1 kernel's
    # h-alternation measured ~210ns/mm effective vs 216 serialized).
    for kind in sched:
        if kind == "D":
            g, step = ds // 64, ds % 64
            ds += 1
            w, c = step % 2, step // 2
            absd = absd_pool.tile([P, M], fp16, name="absd", tag="absd")
            nc.vector.tensor_scalar(
                absd[:], vT2[:], k2T[:, g, w, c:c + 1], None, op0=Alu.max)
            emit_step2(g, w, c, absd)
        else:
            w, c = ss % 2, ss // 2
            emit_step2(NB - 1, w, c, absd_a_tiles[ss])
            ss += 1

    # ---- softmax numerators ----------------------------------------------
    # Banks 0..2 close inside the stream so their exps overlap it; bank
    # 3's exp is the only one that trails, chunked by qp-halves so the
    # first g=3 output matmuls start after half the exp.
    emit_exp(0)
    emit_exp(1)
    emit_exp(2)
    emit_exp(3, chunks=2)

    # ---- weighted sum + denominator via augmented-ones column ------------
    # g = 3 accumulated last within each group: bank 3's exp is the only
    # one that trails the stream, so 12 of the 16 matmuls run before it.
    for qp in range(NB):
        for g in (0, 1, 2, 3):
            nc.tensor.matmul(
                out_all[:, qp, :], wts[g][:, qp * P:(qp + 1) * P],
                v_aug[g][:], start=(g == 0), stop=(g == 3),
                skip_group_check=True)

    # ---- normalize per half + fat DMA out --------------------------------
    # res[p, s, :] = out row 4p+s -> 1KB contiguous per partition.
    recip = small.tile([P, NB], f32, name="recip")
    res = small.tile([P, NB, D], f32, name="res")
    out_v = out.rearrange("(p s) d -> p s d", p=P)
    for h in range(2):
        sl = slice(2 * h, 2 * h + 2)
        nc.vector.reciprocal(recip[:, sl], out_all[:, sl, D])
        rb = recip[:, sl].unsqueeze(2).broadcast_to((P, 2, D))
        nc.vector.tensor_tensor(res[:, sl, :], out_all[:, sl, 0:D], rb,
                                op=Alu.mult)
        nc.sync.dma_start(out_v[:, sl, :], res[:, sl, :])

    if dbg is not None:
        vT2f = small.tile([P, M], f32, name="vT2f")
        nc.vector.tensor_copy(vT2f[:], vT2[:])
        nc.sync.dma_start(dbg["vT2"].ap()[0:P, :], vT2f[:])
        nc.sync.dma_start(dbg["k2T"].ap(),
                          k2T[:].rearrange("p g w c -> p (g w c)"))
        for g in range(NB):
            ut = small.tile([P, M], f32, name=f"unn_dbg_{g}")
            nc.vector.tensor_copy(ut[:], unns[g][:])
            nc.sync.dma_start(dbg["unn"].ap()[g], ut[:])

    ctx.close()


def _get_module():
    if "nc" not in _CACHE:
        _CACHE["nc"] = _build_module()
    return _CACHE["nc"]


def _run(k, v, trace=False, tmpdir=None):
    """k, v: [B, M, D] f32. Returns (out [B, M, D] f32, BassKernelResults)."""
    from concourse import bass_utils

    nc = _get_module()
    kw = {"tmpdir": tmpdir} if tmpdir else {}
    in_maps = [
        {"k": np.ascontiguousarray(k[b], dtype=np.float32),
         "v": np.ascontiguousarray(v[b], dtype=np.float32)}
        for b in range(B)
    ]
    res = bass_utils.run_bass_kernel_spmd(
        nc, in_maps, core_ids=list(range(B)), trace=trace, **kw)
    out = np.stack([res.results[b]["out"] for b in range(B)], axis=0)
    return out, res


def kernel(**inputs):
    k = np.asarray(inputs["k"])
    v = np.asarray(inputs["v"])
    trace = bool(int(os.environ.get("KERNEL_TRACE", "0")))
    try:
        out, _ = _run(k, v, trace=trace)
    except Exception:
        # transient device hiccups happen; one retry on a fresh attempt
        out, _ = _run(k, v, trace=trace)
    return out.astype(np.float32)
